# revision 1
# baseline (speedup 1.0000x reference)
"""MobileMamba block kernel for 8x Trainium2 NeuronCores — chunk-major v2.

Math restructure of the reference:
  xc   = silu(x @ w1.T + b1)                          # [E, L] (channel-major)
  c    = depthwise_conv5(xc) (+bd, BN affine folded)  # [E, L]
  xl   = silu(c)                                      # BN folded into taps/bias
  SSM with constant B/C collapses to a scalar first-order recurrence:
    g[e,t] = expA[e]*g[e,t-1] + xl[e,t]
    ys[e,t] = CB[e]*g[e,t] + Dv[e]*xl[e,t],  CB = sum_s Bm*Cm
  out  = ys @ w2.T + b2   (CB/Dv folded into pre-scaled w2.T copy w2dv)

Sharding: data-parallel over batch (B=8 -> 8 cores). Each core computes one
sample entirely in [channel, time] layout; the host pre-transposes x shards
and post-transposes outputs.

v2 changes vs the previous baseline (m-major):
  * Chunk-major software pipeline with a depth-first first chunk: for each
    512-col time chunk the PE runs [mm1(lc+1) | conv(lc) | mm2(lc-1)] back
    to back, so the mm2 block no longer serializes at the end.
  * All constants (incl. the 20 diag tap matrices) are precomputed on the
    host and arrive via ordered DMAs (md1 m-major with a 64KB head so
    mm1(0) unblocks first); GpSimd no longer spends ~17us building them.
  * The scan's decay operand is a stride-0 broadcast AP (no 512KB expA
    broadcast tensor).
  * 8 zeroed junk matmuls at kernel start warm the PE HAM clock-gate to
    2.4 GHz while the first DMAs are in flight.
  * The TileContext exit barrier is slimmed (_trim_epilogue): per-engine
    drains replaced by NoOps carrying the barrier bookkeeping, second
    barrier dropped (sound under NRT's all-engines-restart semantics).
  * bf16 output DMA (host upcasts); ~0.3% quantization vs 2e-2 tolerance.

Engines: mm1/conv(5 diag taps)/mm2 on TensorE (bf16), silu1/silu2/out-copy
on ScalarE, scan (tensor_tensor_scan, carry chained through the previous
chunk's last column) + fold (STT) on VectorE.  Measured pole: VectorE's
~32us scan+fold stream; ScalarE ~29us; TensorE ~31us of matmul columns.
"""

import sys

for _p in ('/opt/trn_rl_repo',):
    if _p not in sys.path:
        sys.path.append(_p)

import numpy as np

import concourse.bass as bass
import concourse.tile as tile
from concourse import mybir

D = 256      # model dim
E = 512      # expanded dim
L = 2048     # sequence length
B = 8        # batch
NCORES = 8
BN_EPS = 1e-5

F32 = mybir.dt.float32
BF16 = mybir.dt.bfloat16

EM = E // 128   # 4 channel tiles
DM = D // 128   # 2 model-dim tiles

CH = 512
LC = L // CH
TAPS = (0, -1, 1, -2, 2)   # center first: start=True covers full range

# param-table columns (per channel-tile): conv/bn bias, b1, CB/Dv, expA
PT_CBIAS = 0
PT_B1 = 1
PT_CBDV = 2
PT_EXPA = 3
PT_NCOL = 4
MP_COLS = EM * PT_NCOL + DM   # + b2 per d-tile

MD1_COLS = DM * 512            # w1t chunks (bf16)
MD_COLS = EM * 256             # w2dv (bf16)
MDG_COLS = EM * 5 * 128        # diag tap matrices (bf16)
MEA_COLS = EM                  # expA per tile, bf16 (scan data0, bcast AP)


def _bcast(col_ap, n):
    """Broadcast a [128,1] per-partition column AP along the free dim."""
    return bass.AP(tensor=col_ap.tensor, offset=col_ap.offset,
                   ap=[col_ap.ap[0], [0, n]])

# channel tiles whose fold runs as [ACT mult (scale-AP) + DVE 2x TT-add]
# instead of a 1x DVE STT.  Measured: ACT saturates (41us) — keep ().
FOLD_ACT_TILES = ()


def build_nc(wsplit=True, warm=True):
    nc = bass.Bass()
    xt = nc.declare_dram_parameter("xt", [D, L], BF16, isOutput=False)
    md1 = nc.declare_dram_parameter("md1", [128, MD1_COLS], BF16, isOutput=False)
    md = nc.declare_dram_parameter("md", [128, MD_COLS], BF16, isOutput=False)
    mdg = nc.declare_dram_parameter("mdg", [128, MDG_COLS], BF16, isOutput=False)
    mea = nc.declare_dram_parameter("mea", [128, MEA_COLS], BF16, isOutput=False)
    mp = nc.declare_dram_parameter("mp", [128, MP_COLS], F32, isOutput=False)
    # bf16 output halves the out-DMA traffic; the host upcasts.  Quantization
    # adds ~0.3% relative-of-value error vs the 2e-2 tolerance.
    outT = nc.declare_dram_parameter("outT", [D, L], BF16, isOutput=True)

    with tile.TileContext(nc) as tc:
        with (
            tc.tile_pool(name="const", bufs=1) as const,
            tc.tile_pool(name="acts", bufs=1) as acts,
            tc.tile_pool(name="psA", bufs=3, space="PSUM") as psA,
            tc.tile_pool(name="psB", bufs=3, space="PSUM") as psB,
            tc.tile_pool(name="psC", bufs=2, space="PSUM") as psC,
        ):
            # ---- PE warm-up: HAM clock-gates PE at 1.2 GHz until it sees
            # ~3.4us of sustained activity. Junk matmuls on an untracked
            # scratch tensor (no deps at all) run while the first DMAs are
            # in flight so the real mm1 stream starts at 2.4 GHz.
            if warm:
                # Junk MMs on a zeroed scratch tile warm the PE's activity
                # monitor (HAM) to 2.4 GHz while the first DMAs are in
                # flight.  The memset matters: garbage bf16 can hold
                # NaN/Inf patterns that poison PSUM banks.
                warm_src = const.tile([128, CH], BF16)
                nc.vector.memset(warm_src[:, :], 0.0)
                ps_w = psA.tile([128, CH], F32, name="warm", tag="ps1")
                for _ in range(8):
                    nc.tensor.matmul(out=ps_w, lhsT=warm_src[:, 0:128],
                                     rhs=warm_src[:, :], start=True, stop=True)

            # ---- constants + x, issued in order of first use so the first
            # mm1/conv chunks aren't stuck behind later constants.
            # md1 is m-major ([k0-m | k1-m] blocks of 256 cols per m), so a
            # 64KB head DMA unblocks mm1(m=0) almost immediately.
            mw_t = const.tile([128, MD1_COLS], BF16)
            nc.sync.dma_start(out=mw_t[:, 0:256], in_=md1[:, 0:256])

            xts = [acts.tile([128, L], BF16, name=f"xts{k}", tag=f"xt{k}")
                   for k in range(DM)]
            mdg_t = const.tile([128, MDG_COLS], BF16)
            mp_t = const.tile([128, MP_COLS], F32)
            mea_t = const.tile([128, MEA_COLS], BF16)
            md_t = const.tile([128, MD_COLS], BF16)

            def _x_chunk(lc):
                for k in range(DM):
                    nc.sync.dma_start(
                        out=xts[k][:, lc * CH:(lc + 1) * CH],
                        in_=xt[k * 128:(k + 1) * 128, lc * CH:(lc + 1) * CH])

            _x_chunk(0)
            _x_chunk(1)
            # diag taps per channel tile (conv(m, chunk0) needs only tile m)
            MG = 5 * 128
            nc.sync.dma_start(out=mdg_t[:, 0:MG], in_=mdg[:, 0:MG])
            nc.sync.dma_start(out=mw_t[:, 256:], in_=md1[:, 256:])
            nc.sync.dma_start(out=mp_t, in_=mp[:, :])
            nc.sync.dma_start(out=mea_t, in_=mea[:, :])
            nc.sync.dma_start(out=mdg_t[:, MG:2 * MG], in_=mdg[:, MG:2 * MG])
            nc.sync.dma_start(out=mdg_t[:, 2 * MG:], in_=mdg[:, 2 * MG:])
            _x_chunk(2)
            nc.sync.dma_start(out=md_t, in_=md[:, :])
            _x_chunk(3)

            # ---- constant slices (md1 m-major: [k0-m | k1-m] per m) ----
            w1s = [[mw_t[:, m * 256 + k * 128:m * 256 + (k + 1) * 128]
                    for m in range(EM)] for k in range(DM)]
            diag = [[mdg_t[:, (m * 5 + j) * 128:(m * 5 + j + 1) * 128]
                     for j in range(5)] for m in range(EM)]
            w2dvs = [md_t[:, ec * 256:(ec + 1) * 256] for ec in range(EM)]
            cbias_c = [mp_t[:, m * PT_NCOL + PT_CBIAS:m * PT_NCOL + PT_CBIAS + 1]
                       for m in range(EM)]
            b1_c = [mp_t[:, m * PT_NCOL + PT_B1:m * PT_NCOL + PT_B1 + 1]
                    for m in range(EM)]
            cbdv_c = [mp_t[:, m * PT_NCOL + PT_CBDV:m * PT_NCOL + PT_CBDV + 1]
                      for m in range(EM)]
            b2_c = [mp_t[:, EM * PT_NCOL + dt_:EM * PT_NCOL + dt_ + 1]
                    for dt_ in range(DM)]

            xc = [acts.tile([128, L], BF16, name=f"xc{m}", tag=f"xc{m}")
                  for m in range(EM)]
            xl = [acts.tile([128, L], BF16, name=f"xl{m}", tag=f"xl{m}")
                  for m in range(EM)]
            g = [acts.tile([128, L], BF16, name=f"g{m}", tag=f"g{m}")
                 for m in range(EM)]
            gp = [acts.tile([128, L], BF16, name=f"gp{m}", tag=f"gp{m}")
                  for m in range(EM)]
            tmp = {m: acts.tile([128, L], BF16, name=f"tm{m}", tag=f"tm{m}")
                   for m in FOLD_ACT_TILES}
            osb = [acts.tile([128, L], BF16, name=f"o{dt_}", tag=f"o{dt_}")
                   for dt_ in range(DM)]

            def mm1_stage(m, lc):
                c0, c1 = lc * CH, (lc + 1) * CH
                ps1 = psA.tile([128, CH], F32, name="ps1", tag="ps1")
                for k in range(DM):
                    nc.tensor.matmul(
                        out=ps1,
                        lhsT=w1s[k][m],
                        rhs=xts[k][:, c0:c1],
                        start=(k == 0), stop=(k == DM - 1))
                nc.scalar.activation(
                    out=xc[m][:, c0:c1], in_=ps1,
                    func=mybir.ActivationFunctionType.Silu,
                    bias=b1_c[m], scale=1.0)

            def conv_stage(m, a0, b0):
                n = b0 - a0
                ps2 = psB.tile([128, CH], F32, name="ps2", tag="ps2")
                for j, dlt in enumerate(TAPS):
                    lo, hi = max(0, -dlt), L - max(0, dlt)
                    a, b_ = max(a0, lo), min(b0, hi)
                    if a >= b_:
                        continue
                    nc.tensor.matmul(
                        out=ps2[:, a - a0:b_ - a0],
                        lhsT=diag[m][j],
                        rhs=xc[m][:, a + dlt:b_ + dlt],
                        start=(j == 0), stop=(j == len(TAPS) - 1),
                        skip_group_check=True)
                nc.scalar.activation(
                    out=xl[m][:, a0:b0], in_=ps2[:, 0:n],
                    func=mybir.ActivationFunctionType.Silu,
                    bias=cbias_c[m], scale=1.0)

            def scan_stage(m, a0, b0):
                n = b0 - a0
                nc.vector.tensor_tensor_scan(
                    out=g[m][:, a0:b0], data0=_bcast(mea_t[:, m:m + 1], n),
                    data1=xl[m][:, a0:b0],
                    initial=(0.0 if a0 == 0 else g[m][:, a0 - 1:a0]),
                    op0=mybir.AluOpType.mult, op1=mybir.AluOpType.add)

            def fold_mult_stage(m, a0, b0):
                # ACT half of the fold: tmp = cbdv * g (per-partition scale)
                nc.scalar.activation(
                    out=tmp[m][:, a0:b0], in_=g[m][:, a0:b0],
                    func=mybir.ActivationFunctionType.Identity,
                    bias=0.0, scale=cbdv_c[m])

            def fold_stage(m, a0, b0):
                if m in FOLD_ACT_TILES:
                    nc.vector.tensor_tensor(
                        out=gp[m][:, a0:b0], in0=tmp[m][:, a0:b0],
                        in1=xl[m][:, a0:b0], op=mybir.AluOpType.add)
                else:
                    nc.vector.scalar_tensor_tensor(
                        out=gp[m][:, a0:b0], in0=g[m][:, a0:b0],
                        scalar=cbdv_c[m], in1=xl[m][:, a0:b0],
                        op0=mybir.AluOpType.mult, op1=mybir.AluOpType.add)

            def mm2_stage(a0, b0, dma=True):
                n = b0 - a0
                for dt_ in range(DM):
                    ps3 = psC.tile([128, CH], F32, name="ps3", tag="ps3")
                    for ec in range(EM):
                        nc.tensor.matmul(
                            out=ps3[:, 0:n],
                            lhsT=w2dvs[ec][:, dt_ * 128:(dt_ + 1) * 128],
                            rhs=gp[ec][:, a0:b0],
                            start=(ec == 0), stop=(ec == EM - 1))
                    nc.scalar.activation(
                        out=osb[dt_][:, a0:b0], in_=ps3[:, 0:n],
                        func=mybir.ActivationFunctionType.Identity,
                        bias=b2_c[dt_], scale=1.0)
                    if dma:
                        nc.sync.dma_start(
                            out=outT[dt_ * 128:(dt_ + 1) * 128, a0:b0],
                            in_=osb[dt_][:, a0:b0])

            def scan_fold_block(a0, b0):
                # DVE stream: scan0, fold0, scan1, fold1, scan2, scan3,
                # add2, add3 — the TT-adds (which wait on ACT's tmp mults)
                # come after every scan so DVE never stalls mid-stream.
                for m in range(EM):
                    scan_stage(m, a0, b0)
                    if m in FOLD_ACT_TILES:
                        fold_mult_stage(m, a0, b0)
                    else:
                        fold_stage(m, a0, b0)
                for m in FOLD_ACT_TILES:
                    fold_stage(m, a0, b0)

            # ---- chunk-major schedule ----
            # conv lags mm1 by one chunk (right halo = first 2 cols of the
            # next chunk); mm2 lags the tail stages by one more chunk.
            # The first iteration runs depth-first per tile so the scan
            # (VectorE, the pipeline pole) starts as early as possible.
            for m in range(EM):
                mm1_stage(m, 0)
                mm1_stage(m, 1)
                conv_stage(m, 0, CH)
            scan_fold_block(0, CH)
            for lc in range(1, LC):
                if lc + 1 < LC:
                    for m in range(EM):
                        mm1_stage(m, lc + 1)
                for m in range(EM):
                    conv_stage(m, lc * CH, (lc + 1) * CH)
                scan_fold_block(lc * CH, (lc + 1) * CH)
                mm2_stage((lc - 1) * CH, lc * CH)
            # last chunk's mm2 in halves so the first half's copy+DMA
            # overlaps the second half's matmuls
            HF = CH // 2
            for s0 in range((LC - 1) * CH, L, HF):
                mm2_stage(s0, s0 + HF)

    _trim_epilogue(nc)
    if wsplit:
        _split_waits(nc)
    return nc


def _trim_epilogue(nc):
    """Slim the TileContext exit sequence inside the timed window.

    The stock epilogue is [SP drain w/ DMA waits | barrier1 (drain+sem per
    engine) | Pool sem/dma range-clear | barrier2 (drain+sem per engine)].
    The per-engine InstDrains and the whole second barrier cost ~4-6us of
    serialized wall time.  Engines execute in order, so by the time each
    engine's barrier1 EventSemaphore runs its prior work has completed; the
    only async completions are DMAs, which the kept SP drain waits for.  NRT
    restarts all engines together on a re-execute, so nothing can race the
    Pool range-clear once barrier1 has passed — barrier2 is redundant.
    """
    for f in nc.m.functions:
        for bb in f.blocks:
            if not bb.name.endswith("_end"):
                continue
            out = []
            first_drain = True
            seen_isa = False
            for inst in bb.instructions:
                cn = inst.__class__.__name__
                if cn == "InstDrain":
                    si = inst.sync_info
                    if first_drain and si and si.on_wait:
                        out.append(inst)   # SP drain carrying DMA-clock waits
                    elif getattr(inst, "is_reset_sema", False):
                        out.append(inst)   # Pool dma_reset (per-sem DMA state)
                    elif si and (si.on_update or si.on_wait):
                        # keep the barrier bookkeeping (gather++) minus the
                        # expensive engine quiesce
                        out.append(mybir.InstNoOp(
                            name=f"{inst.name}_nodrain", engine=inst.engine,
                            sync_info=si))
                    first_drain = False
                    continue
                if cn == "InstISA":
                    seen_isa = True
                    out.append(inst)
                    continue
                if cn == "InstEventSemaphore" and seen_isa:
                    continue               # barrier2 sems
                out.append(inst)
            bb.instructions = out
    return nc


_WSPLIT_SKIP = ("InstAllEngineBarrier", "InstNoOp",
                "InstEventSemaphore", "InstUnconditionalBranch")


def _split_waits(nc, max_waits=1):
    """Walrus codegen allows a single sync-wait command per TPB instruction.

    Move all-but-one waits of any over-limit instruction onto preceding
    NoOps (one wait each) on the same engine; same-engine program order
    makes this sound.
    """
    n_split = 0
    for f in nc.m.functions:
        for bb in f.blocks:
            out = []
            for inst in bb.instructions:
                si = inst.sync_info
                waits = list(si.on_wait) if si and si.on_wait else []
                if (len(waits) > max_waits
                        and inst.__class__.__name__ not in _WSPLIT_SKIP):
                    spill, keep = waits[:-max_waits], waits[-max_waits:]
                    for i, w in enumerate(spill):
                        out.append(mybir.InstNoOp(
                            name=f"{inst.name}_ws{i}",
                            engine=inst.engine,
                            sync_info=mybir.SyncInfo(on_wait=[w],
                                                     on_update=[]),
                        ))
                        n_split += 1
                    si.on_wait = keep
                out.append(inst)
            if n_split:
                bb.instructions = out
    return nc


def _to_bf16(a):
    import ml_dtypes
    return a.astype(ml_dtypes.bfloat16)


def host_params(w1, b1, wd, bd, gamma, beta, rmean, rvar, A, Bm, Cm, Dv, w2, b2):
    s = (gamma / np.sqrt(rvar + BN_EPS)).astype(np.float32)
    cw = (wd[:, 0, :] * s[:, None]).astype(np.float32)            # [E, 5]
    cbias = (bd * s + beta - rmean * s).astype(np.float32)        # [E]
    expA = np.exp(A).astype(np.float32)                           # [E]
    CB = (Bm * Cm).sum(1).astype(np.float32)                      # [E]
    w1t = np.asarray(w1, np.float32).T                            # [D, E]
    w2t = np.asarray(w2, np.float32).T                            # [E, D]

    # m-major: per channel-tile m, [k0 block | k1 block] of 128 cols each
    md1 = np.zeros((128, MD1_COLS), np.float32)
    for m in range(EM):
        for k in range(DM):
            md1[:, m * 256 + k * 128:m * 256 + (k + 1) * 128] = \
                w1t[k * 128:(k + 1) * 128, m * 128:(m + 1) * 128]

    dv = np.asarray(Dv, np.float32).copy()
    tiny = np.abs(dv) < 1e-6
    dv[tiny] = np.where(dv[tiny] < 0, -1e-6, 1e-6)
    cbdv = CB / dv

    mdm = np.zeros((128, MD_COLS), np.float32)
    for ec in range(EM):
        blk = w2t[ec * 128:(ec + 1) * 128, :]
        mdm[:, ec * 256:(ec + 1) * 256] = blk * dv[ec * 128:(ec + 1) * 128, None]

    # diag tap matrices: diag[p, f] = cw_j[p] if f == p else 0; TAPS order
    mdg = np.zeros((128, MDG_COLS), np.float32)
    for m in range(EM):
        for j, dlt in enumerate(TAPS):
            c0 = (m * 5 + j) * 128
            np.fill_diagonal(mdg[:, c0:c0 + 128],
                             cw[m * 128:(m + 1) * 128, dlt + 2])

    # expA per tile (scan data0 via stride-0 broadcast AP)
    mea = np.zeros((128, MEA_COLS), np.float32)
    for m in range(EM):
        mea[:, m] = expA[m * 128:(m + 1) * 128]

    mpm = np.zeros((128, MP_COLS), np.float32)
    for m in range(EM):
        sl = slice(m * 128, (m + 1) * 128)
        mpm[:, m * PT_NCOL + PT_CBIAS] = cbias[sl]
        mpm[:, m * PT_NCOL + PT_B1] = np.asarray(b1, np.float32)[sl]
        mpm[:, m * PT_NCOL + PT_CBDV] = cbdv[sl]
        mpm[:, m * PT_NCOL + PT_EXPA] = expA[sl]
    for dt_ in range(DM):
        mpm[:, EM * PT_NCOL + dt_] = \
            np.asarray(b2, np.float32)[dt_ * 128:(dt_ + 1) * 128]

    return dict(md1=_to_bf16(md1), md=_to_bf16(mdm), mdg=_to_bf16(mdg),
                mea=_to_bf16(mea), mp=mpm)


_CACHED_NC = None


def kernel(x, w1, b1, wd, bd, gamma, beta, rmean, rvar, A, Bm, Cm, Dv, w2, b2,
           **run_kwargs):
    from concourse.bass_utils import run_bass_kernel_spmd
    global _CACHED_NC
    if _CACHED_NC is None:
        _CACHED_NC = build_nc()
    nc = _CACHED_NC

    params = host_params(w1, b1, wd, bd, gamma, beta, rmean, rvar,
                         A, Bm, Cm, Dv, w2, b2)
    x = np.asarray(x, dtype=np.float32)
    in_maps = []
    for i in range(NCORES):
        m = dict(params)
        m["xt"] = _to_bf16(np.ascontiguousarray(x[i].T))  # [D, L] bf16
        in_maps.append(m)

    res = run_bass_kernel_spmd(nc, in_maps, core_ids=list(range(NCORES)),
                               **run_kwargs)
    out = np.stack([np.asarray(r["outT"], dtype=np.float32).T
                    for r in res.results])                          # [B, L, D]
    if run_kwargs:
        kernel.last_result = res
    return out



# revision 7
# speedup vs baseline: 1.0204x; 1.0204x over previous
"""MobileMamba block kernel for 8x Trainium2 NeuronCores — chunk-major v2.

Math restructure of the reference:
  xc   = silu(x @ w1.T + b1)                          # [E, L] (channel-major)
  c    = depthwise_conv5(xc) (+bd, BN affine folded)  # [E, L]
  xl   = silu(c)                                      # BN folded into taps/bias
  SSM with constant B/C collapses to a scalar first-order recurrence:
    g[e,t] = expA[e]*g[e,t-1] + xl[e,t]
    ys[e,t] = CB[e]*g[e,t] + Dv[e]*xl[e,t],  CB = sum_s Bm*Cm
  out  = ys @ w2.T + b2   (CB/Dv folded into pre-scaled w2.T copy w2dv)

Sharding: data-parallel over batch (B=8 -> 8 cores). Each core computes one
sample entirely in [channel, time] layout; the host pre-transposes x shards
and post-transposes outputs.

v3 changes vs v2 (which measured 58.4us):
  * Conv/scan/fold/mm2 chunks shifted -128 vs the mm1 grid
    (KB = 0,384,896,1408,1920,2048): conv K0 needs only mm1 chunk0, so the
    DVE scan stream starts ~6us earlier (v2 waited for mm1 chunks 0 AND 1
    of all four tiles before the first conv).
  * mp (bias tables) DMA'd right after xt chunk0 — v2 issued it 5th and
    the first silu1 stalled 2.2us waiting for it.
  * A dummy 1-col SILU at kernel start pulls the 1.28us ACT_TABLE_LOAD
    off the critical path.
  * Warm-up is 28 64-col junk matmuls (drain <100ns each) instead of
    8x512 (3.2us of in-order PE queue ahead of the first real mm1).
  * 128-col final chunk + split mm2 tail: only the last ec matmul pair
    gates on the final fold, then copy + a small out-DMA.

Engines: mm1/conv(5 diag taps)/mm2 on TensorE (bf16), silu1/silu2/out-copy
on ScalarE, scan (tensor_tensor_scan, carry chained through the previous
chunk's last column) + fold (STT) on VectorE.  Streams: VectorE ~32us,
TensorE ~31us (74k matmul cols at 2.4GHz), ScalarE ~29us.
"""

import sys

for _p in ('/opt/trn_rl_repo',):
    if _p not in sys.path:
        sys.path.append(_p)

import numpy as np

import concourse.bass as bass
import concourse.tile as tile
from concourse import mybir

D = 256      # model dim
E = 512      # expanded dim
L = 2048     # sequence length
B = 8        # batch
NCORES = 8
BN_EPS = 1e-5

F32 = mybir.dt.float32
BF16 = mybir.dt.bfloat16

EM = E // 128   # 4 channel tiles
DM = D // 128   # 2 model-dim tiles

CH = 512
LC = L // CH
# conv/scan/fold/mm2 chunk boundaries, shifted -128 vs the mm1 grid so the
# first conv chunk [0,384) needs only mm1 chunk0 (no right-halo wait on
# chunk1) and the last chunk is a short 128-col tail.
KB = (0, 384, 896, 1408, 1920, 2048)
NK = len(KB) - 1
TAPS = (0, -1, 1, -2, 2)   # center first: start=True covers full range

# param-table columns (per channel-tile): conv/bn bias, b1, CB/Dv, expA
PT_CBIAS = 0
PT_B1 = 1
PT_CBDV = 2
PT_EXPA = 3
PT_NCOL = 4
MP_COLS = EM * PT_NCOL + DM   # + b2 per d-tile

MD1_COLS = DM * 512            # w1t chunks (bf16)
MD_COLS = EM * 256             # w2dv (bf16)
MDG_COLS = EM * 5 * 128        # diag tap matrices (bf16)
MEA_COLS = EM                  # expA per tile, bf16 (scan data0, bcast AP)


def _bcast(col_ap, n):
    """Broadcast a [128,1] per-partition column AP along the free dim."""
    return bass.AP(tensor=col_ap.tensor, offset=col_ap.offset,
                   ap=[col_ap.ap[0], [0, n]])


def build_nc(wsplit=True, warm=True):
    nc = bass.Bass()
    xt = nc.declare_dram_parameter("xt", [D, L], BF16, isOutput=False)
    md1 = nc.declare_dram_parameter("md1", [128, MD1_COLS], BF16, isOutput=False)
    md = nc.declare_dram_parameter("md", [128, MD_COLS], BF16, isOutput=False)
    mdg = nc.declare_dram_parameter("mdg", [128, MDG_COLS], BF16, isOutput=False)
    mea = nc.declare_dram_parameter("mea", [128, MEA_COLS], BF16, isOutput=False)
    mp = nc.declare_dram_parameter("mp", [128, MP_COLS], F32, isOutput=False)
    # bf16 output halves the out-DMA traffic; the host upcasts.  Quantization
    # adds ~0.3% relative-of-value error vs the 2e-2 tolerance.
    outT = nc.declare_dram_parameter("outT", [D, L], BF16, isOutput=True)

    with tile.TileContext(nc) as tc:
        with (
            tc.tile_pool(name="const", bufs=1) as const,
            tc.tile_pool(name="acts", bufs=1) as acts,
            tc.tile_pool(name="psA", bufs=3, space="PSUM") as psA,
            tc.tile_pool(name="psB", bufs=3, space="PSUM") as psB,
            tc.tile_pool(name="psC", bufs=2, space="PSUM") as psC,
        ):
            # ---- PE warm-up: HAM clock-gates the PE until it sees ~3us of
            # sustained activity.  Small 64-col junk matmuls keep the
            # activity monitor fed while the first DMAs are in flight but
            # drain in <100ns each, so the real mm1 stream isn't stuck
            # behind a fat junk queue (the old 8x512 warm-up cost 3.2us of
            # in-order PE queue time).
            warm_src = const.tile([128, CH], BF16)
            if warm:
                nc.vector.memset(warm_src[:, :], 0.0)
                ps_w = psA.tile([128, CH], F32, name="warm", tag="ps1")
                for _ in range(28):
                    nc.tensor.matmul(out=ps_w[:, 0:64], lhsT=warm_src[:, 0:128],
                                     rhs=warm_src[:, 0:64], start=True, stop=True)

            # Dummy 1-col SILU pulls the 1.28us ACT_TABLE_LOAD off the
            # critical path (it otherwise runs lazily right before the first
            # real silu1).
            dmy_t = const.tile([128, 1], F32)
            nc.scalar.activation(
                out=dmy_t[:, 0:1], in_=warm_src[:, 0:1],
                func=mybir.ActivationFunctionType.Silu, bias=0.0, scale=1.0)

            # ---- constants + x, issued in order of first use so the first
            # mm1/conv chunks aren't stuck behind later constants.
            # md1 is m-major ([k0-m | k1-m] blocks of 256 cols per m), so a
            # 64KB head DMA unblocks mm1(m=0) almost immediately.  mp (bias
            # tables) is issued right after xt chunk0: silu1 needs it.
            mw_t = const.tile([128, MD1_COLS], BF16)
            nc.sync.dma_start(out=mw_t[:, 0:256], in_=md1[:, 0:256])

            xts = [acts.tile([128, L], BF16, name=f"xts{k}", tag=f"xt{k}")
                   for k in range(DM)]
            mdg_t = const.tile([128, MDG_COLS], BF16)
            mp_t = const.tile([128, MP_COLS], F32)
            mea_t = const.tile([128, MEA_COLS], BF16)
            md_t = const.tile([128, MD_COLS], BF16)

            def _x_chunk(lc):
                for k in range(DM):
                    nc.sync.dma_start(
                        out=xts[k][:, lc * CH:(lc + 1) * CH],
                        in_=xt[k * 128:(k + 1) * 128, lc * CH:(lc + 1) * CH])

            MG = 5 * 128
            _x_chunk(0)
            nc.sync.dma_start(out=mp_t, in_=mp[:, :])
            # diag taps per channel tile (conv(m, chunk0) needs only tile m)
            nc.sync.dma_start(out=mdg_t[:, 0:MG], in_=mdg[:, 0:MG])
            nc.sync.dma_start(out=mea_t, in_=mea[:, :])
            _x_chunk(1)
            nc.sync.dma_start(out=mdg_t[:, MG:2 * MG], in_=mdg[:, MG:2 * MG])
            nc.sync.dma_start(out=mw_t[:, 256:], in_=md1[:, 256:])
            nc.sync.dma_start(out=mdg_t[:, 2 * MG:], in_=mdg[:, 2 * MG:])
            _x_chunk(2)
            nc.sync.dma_start(out=md_t, in_=md[:, :])
            _x_chunk(3)

            # ---- constant slices (md1 m-major: [k0-m | k1-m] per m) ----
            w1s = [[mw_t[:, m * 256 + k * 128:m * 256 + (k + 1) * 128]
                    for m in range(EM)] for k in range(DM)]
            diag = [[mdg_t[:, (m * 5 + j) * 128:(m * 5 + j + 1) * 128]
                     for j in range(5)] for m in range(EM)]
            w2dvs = [md_t[:, ec * 256:(ec + 1) * 256] for ec in range(EM)]
            cbias_c = [mp_t[:, m * PT_NCOL + PT_CBIAS:m * PT_NCOL + PT_CBIAS + 1]
                       for m in range(EM)]
            b1_c = [mp_t[:, m * PT_NCOL + PT_B1:m * PT_NCOL + PT_B1 + 1]
                    for m in range(EM)]
            cbdv_c = [mp_t[:, m * PT_NCOL + PT_CBDV:m * PT_NCOL + PT_CBDV + 1]
                      for m in range(EM)]
            b2_c = [mp_t[:, EM * PT_NCOL + dt_:EM * PT_NCOL + dt_ + 1]
                    for dt_ in range(DM)]

            xc = [acts.tile([128, L], BF16, name=f"xc{m}", tag=f"xc{m}")
                  for m in range(EM)]
            xl = [acts.tile([128, L], BF16, name=f"xl{m}", tag=f"xl{m}")
                  for m in range(EM)]
            g = [acts.tile([128, L], BF16, name=f"g{m}", tag=f"g{m}")
                 for m in range(EM)]
            gp = [acts.tile([128, L], BF16, name=f"gp{m}", tag=f"gp{m}")
                  for m in range(EM)]
            osb = [acts.tile([128, L], BF16, name=f"o{dt_}", tag=f"o{dt_}")
                   for dt_ in range(DM)]

            def mm1_stage(m, lc):
                c0, c1 = lc * CH, (lc + 1) * CH
                ps1 = psA.tile([128, CH], F32, name="ps1", tag="ps1")
                for k in range(DM):
                    nc.tensor.matmul(
                        out=ps1,
                        lhsT=w1s[k][m],
                        rhs=xts[k][:, c0:c1],
                        start=(k == 0), stop=(k == DM - 1))
                nc.scalar.activation(
                    out=xc[m][:, c0:c1], in_=ps1,
                    func=mybir.ActivationFunctionType.Silu,
                    bias=b1_c[m], scale=1.0)

            def conv_stage(m, a0, b0):
                n = b0 - a0
                ps2 = psB.tile([128, CH], F32, name="ps2", tag="ps2")
                for j, dlt in enumerate(TAPS):
                    lo, hi = max(0, -dlt), L - max(0, dlt)
                    a, b_ = max(a0, lo), min(b0, hi)
                    if a >= b_:
                        continue
                    nc.tensor.matmul(
                        out=ps2[:, a - a0:b_ - a0],
                        lhsT=diag[m][j],
                        rhs=xc[m][:, a + dlt:b_ + dlt],
                        start=(j == 0), stop=(j == len(TAPS) - 1),
                        skip_group_check=True)
                nc.scalar.activation(
                    out=xl[m][:, a0:b0], in_=ps2[:, 0:n],
                    func=mybir.ActivationFunctionType.Silu,
                    bias=cbias_c[m], scale=1.0)

            def scan_stage(m, a0, b0):
                n = b0 - a0
                nc.vector.tensor_tensor_scan(
                    out=g[m][:, a0:b0], data0=_bcast(mea_t[:, m:m + 1], n),
                    data1=xl[m][:, a0:b0],
                    initial=(0.0 if a0 == 0 else g[m][:, a0 - 1:a0]),
                    op0=mybir.AluOpType.mult, op1=mybir.AluOpType.add)

            def fold_stage(m, a0, b0):
                nc.vector.scalar_tensor_tensor(
                    out=gp[m][:, a0:b0], in0=g[m][:, a0:b0],
                    scalar=cbdv_c[m], in1=xl[m][:, a0:b0],
                    op0=mybir.AluOpType.mult, op1=mybir.AluOpType.add)

            def mm2_stage(a0, b0, dma=True):
                n = b0 - a0
                for dt_ in range(DM):
                    ps3 = psC.tile([128, CH], F32, name="ps3", tag="ps3")
                    for ec in range(EM):
                        nc.tensor.matmul(
                            out=ps3[:, 0:n],
                            lhsT=w2dvs[ec][:, dt_ * 128:(dt_ + 1) * 128],
                            rhs=gp[ec][:, a0:b0],
                            start=(ec == 0), stop=(ec == EM - 1))
                    nc.scalar.activation(
                        out=osb[dt_][:, a0:b0], in_=ps3[:, 0:n],
                        func=mybir.ActivationFunctionType.Identity,
                        bias=b2_c[dt_], scale=1.0)
                    if dma:
                        nc.sync.dma_start(
                            out=outT[dt_ * 128:(dt_ + 1) * 128, a0:b0],
                            in_=osb[dt_][:, a0:b0])

            def mm2_tail(a0, b0):
                # Final chunk: pre-run the ec0-2 accumulation for both
                # d-tiles while the last scan/folds stream on DVE; only the
                # ec3 matmuls gate on the final fold.
                n = b0 - a0
                ps3s = []
                for dt_ in range(DM):
                    ps3 = psC.tile([128, CH], F32, name="ps3", tag="ps3")
                    for ec in range(EM - 1):
                        nc.tensor.matmul(
                            out=ps3[:, 0:n],
                            lhsT=w2dvs[ec][:, dt_ * 128:(dt_ + 1) * 128],
                            rhs=gp[ec][:, a0:b0],
                            start=(ec == 0), stop=False,
                            skip_group_check=True)
                    ps3s.append(ps3)
                for dt_ in range(DM):
                    nc.tensor.matmul(
                        out=ps3s[dt_][:, 0:n],
                        lhsT=w2dvs[EM - 1][:, dt_ * 128:(dt_ + 1) * 128],
                        rhs=gp[EM - 1][:, a0:b0],
                        start=False, stop=True, skip_group_check=True)
                    nc.scalar.activation(
                        out=osb[dt_][:, a0:b0], in_=ps3s[dt_][:, 0:n],
                        func=mybir.ActivationFunctionType.Identity,
                        bias=b2_c[dt_], scale=1.0)
                    nc.sync.dma_start(
                        out=outT[dt_ * 128:(dt_ + 1) * 128, a0:b0],
                        in_=osb[dt_][:, a0:b0])

            def scan_fold_block(a0, b0):
                for m in range(EM):
                    scan_stage(m, a0, b0)
                    fold_stage(m, a0, b0)

            def mm1_conv_batch(c):
                # PE order: two mm1s lead so conv(m) never waits on its own
                # silu1 back-to-back; conv(m, K_c) reads xc with a +-2 halo
                # that stays within mm1 chunks <= c.
                mm1_stage(0, c)
                mm1_stage(1, c)
                conv_stage(0, KB[c], KB[c + 1])
                mm1_stage(2, c)
                conv_stage(1, KB[c], KB[c + 1])
                mm1_stage(3, c)
                conv_stage(2, KB[c], KB[c + 1])
                conv_stage(3, KB[c], KB[c + 1])

            # ---- chunk-major schedule ----
            # K-chunks are shifted -128 vs the mm1 grid: conv K0 needs only
            # mm1 c0, so the scan (the DVE stream pole) starts ~6us earlier
            # than with aligned chunks.  mm2 lags the scan/fold by one chunk.
            for c in range(LC):
                mm1_conv_batch(c)
                scan_fold_block(KB[c], KB[c + 1])
                if c >= 1:
                    mm2_stage(KB[c - 1], KB[c])
            for m in range(EM):
                conv_stage(m, KB[LC], KB[LC + 1])
            scan_fold_block(KB[LC], KB[LC + 1])
            mm2_stage(KB[LC - 1], KB[LC])
            mm2_tail(KB[LC], KB[LC + 1])

    _trim_epilogue(nc)
    if wsplit:
        _split_waits(nc)
    return nc


def _trim_epilogue(nc):
    """Slim the TileContext exit sequence inside the timed window.

    The stock epilogue is [SP drain w/ DMA waits | barrier1 (drain+sem per
    engine) | Pool sem/dma range-clear | barrier2 (drain+sem per engine)].
    The per-engine InstDrains and the whole second barrier cost ~4-6us of
    serialized wall time.  Engines execute in order, so by the time each
    engine's barrier1 EventSemaphore runs its prior work has completed; the
    only async completions are DMAs, which the kept SP drain waits for.  NRT
    restarts all engines together on a re-execute, so nothing can race the
    Pool range-clear once barrier1 has passed — barrier2 is redundant.
    """
    for f in nc.m.functions:
        for bb in f.blocks:
            if not bb.name.endswith("_end"):
                continue
            out = []
            first_drain = True
            seen_isa = False
            for inst in bb.instructions:
                cn = inst.__class__.__name__
                if cn == "InstDrain":
                    si = inst.sync_info
                    if first_drain and si and si.on_wait:
                        out.append(inst)   # SP drain carrying DMA-clock waits
                    elif getattr(inst, "is_reset_sema", False):
                        out.append(inst)   # Pool dma_reset (per-sem DMA state)
                    elif si and (si.on_update or si.on_wait):
                        # keep the barrier bookkeeping (gather++) minus the
                        # expensive engine quiesce
                        out.append(mybir.InstNoOp(
                            name=f"{inst.name}_nodrain", engine=inst.engine,
                            sync_info=si))
                    first_drain = False
                    continue
                if cn == "InstISA":
                    seen_isa = True
                    out.append(inst)
                    continue
                if cn == "InstEventSemaphore" and seen_isa:
                    continue               # barrier2 sems
                out.append(inst)
            bb.instructions = out
    return nc


_WSPLIT_SKIP = ("InstAllEngineBarrier", "InstNoOp",
                "InstEventSemaphore", "InstUnconditionalBranch")


def _split_waits(nc, max_waits=1):
    """Walrus codegen allows a single sync-wait command per TPB instruction.

    Move all-but-one waits of any over-limit instruction onto preceding
    NoOps (one wait each) on the same engine; same-engine program order
    makes this sound.
    """
    n_split = 0
    for f in nc.m.functions:
        for bb in f.blocks:
            out = []
            for inst in bb.instructions:
                si = inst.sync_info
                waits = list(si.on_wait) if si and si.on_wait else []
                if (len(waits) > max_waits
                        and inst.__class__.__name__ not in _WSPLIT_SKIP):
                    spill, keep = waits[:-max_waits], waits[-max_waits:]
                    for i, w in enumerate(spill):
                        out.append(mybir.InstNoOp(
                            name=f"{inst.name}_ws{i}",
                            engine=inst.engine,
                            sync_info=mybir.SyncInfo(on_wait=[w],
                                                     on_update=[]),
                        ))
                        n_split += 1
                    si.on_wait = keep
                out.append(inst)
            if n_split:
                bb.instructions = out
    return nc


def _to_bf16(a):
    import ml_dtypes
    return a.astype(ml_dtypes.bfloat16)


def host_params(w1, b1, wd, bd, gamma, beta, rmean, rvar, A, Bm, Cm, Dv, w2, b2):
    s = (gamma / np.sqrt(rvar + BN_EPS)).astype(np.float32)
    cw = (wd[:, 0, :] * s[:, None]).astype(np.float32)            # [E, 5]
    cbias = (bd * s + beta - rmean * s).astype(np.float32)        # [E]
    expA = np.exp(A).astype(np.float32)                           # [E]
    CB = (Bm * Cm).sum(1).astype(np.float32)                      # [E]
    w1t = np.asarray(w1, np.float32).T                            # [D, E]
    w2t = np.asarray(w2, np.float32).T                            # [E, D]

    # m-major: per channel-tile m, [k0 block | k1 block] of 128 cols each
    md1 = np.zeros((128, MD1_COLS), np.float32)
    for m in range(EM):
        for k in range(DM):
            md1[:, m * 256 + k * 128:m * 256 + (k + 1) * 128] = \
                w1t[k * 128:(k + 1) * 128, m * 128:(m + 1) * 128]

    dv = np.asarray(Dv, np.float32).copy()
    tiny = np.abs(dv) < 1e-6
    dv[tiny] = np.where(dv[tiny] < 0, -1e-6, 1e-6)
    cbdv = CB / dv

    mdm = np.zeros((128, MD_COLS), np.float32)
    for ec in range(EM):
        blk = w2t[ec * 128:(ec + 1) * 128, :]
        mdm[:, ec * 256:(ec + 1) * 256] = blk * dv[ec * 128:(ec + 1) * 128, None]

    # diag tap matrices: diag[p, f] = cw_j[p] if f == p else 0; TAPS order
    mdg = np.zeros((128, MDG_COLS), np.float32)
    for m in range(EM):
        for j, dlt in enumerate(TAPS):
            c0 = (m * 5 + j) * 128
            np.fill_diagonal(mdg[:, c0:c0 + 128],
                             cw[m * 128:(m + 1) * 128, dlt + 2])

    # expA per tile (scan data0 via stride-0 broadcast AP)
    mea = np.zeros((128, MEA_COLS), np.float32)
    for m in range(EM):
        mea[:, m] = expA[m * 128:(m + 1) * 128]

    mpm = np.zeros((128, MP_COLS), np.float32)
    for m in range(EM):
        sl = slice(m * 128, (m + 1) * 128)
        mpm[:, m * PT_NCOL + PT_CBIAS] = cbias[sl]
        mpm[:, m * PT_NCOL + PT_B1] = np.asarray(b1, np.float32)[sl]
        mpm[:, m * PT_NCOL + PT_CBDV] = cbdv[sl]
        mpm[:, m * PT_NCOL + PT_EXPA] = expA[sl]
    for dt_ in range(DM):
        mpm[:, EM * PT_NCOL + dt_] = \
            np.asarray(b2, np.float32)[dt_ * 128:(dt_ + 1) * 128]

    return dict(md1=_to_bf16(md1), md=_to_bf16(mdm), mdg=_to_bf16(mdg),
                mea=_to_bf16(mea), mp=mpm)


_CACHED_NC = None


def kernel(x, w1, b1, wd, bd, gamma, beta, rmean, rvar, A, Bm, Cm, Dv, w2, b2,
           **run_kwargs):
    from concourse.bass_utils import run_bass_kernel_spmd
    global _CACHED_NC
    if _CACHED_NC is None:
        _CACHED_NC = build_nc()
    nc = _CACHED_NC

    params = host_params(w1, b1, wd, bd, gamma, beta, rmean, rvar,
                         A, Bm, Cm, Dv, w2, b2)
    x = np.asarray(x, dtype=np.float32)
    in_maps = []
    for i in range(NCORES):
        m = dict(params)
        m["xt"] = _to_bf16(np.ascontiguousarray(x[i].T))  # [D, L] bf16
        in_maps.append(m)

    res = run_bass_kernel_spmd(nc, in_maps, core_ids=list(range(NCORES)),
                               **run_kwargs)
    out = np.stack([np.asarray(r["outT"], dtype=np.float32).T
                    for r in res.results])                          # [B, L, D]
    if run_kwargs:
        kernel.last_result = res
    return out

